# revision 29
# baseline (speedup 1.0000x reference)
"""MultiHeadAttention with RoPE on 8 Trainium2 NeuronCores.

Sharding: batch (2) x head-group (4 heads each) -> 8 cores. Each core
computes q/k/v projections for its 4 heads of one batch element, causal
attention, and a partial output projection (row-shard of Wo). The host
sums the 4 partial outputs per batch element (the "all-reduce").

All matmul operands are bf16 (PSUM accumulates fp32): fp32 HIGH-mode
matmuls disable FWL and run multi-pass; bf16 streams 1 col/cycle.

The attention phase is rate-limited by ACT (exp ~864ns/item vs ~430ns
of PE score+AV work per item), which starves the PE and lets the HAM
clock gate throttle it to 1.2 GHz. Countermeasures:
  - AV matmuls are deferred one work item behind their exp (software
    pipeline) so the PE always has score work queued while ACT runs.
  - The V projection for token blocks 4..15 and the per-q-tile output
    projection are interleaved into the attention stream as PE filler.
PSUM budget: psSC 2x[128,1024] (4 banks) + psAV 2x[128,512] (2) +
psX filler 2x[128,512] (2) = 8 banks.

Device layout per core:
  - x.T (d-major, bf16) streamed to SBUF once (token-major chunks so
    compute starts early); all projections contract over d.
  - Q/K produced channel-partition (Q.T layout); RoPE applied via DVE
    stream_shuffle (partition XOR-1) + cos/sin tables, result cast bf16.
  - scores computed transposed (k on partitions, q on free) so the AV
    matmul needs no transposes; the two heads of a pair go to partition
    bases 0/64 (row groups h0/h64).
  - softmax denominators come free from an extra ones-column in the
    V-stationary AV matmul (M=65); exp on ACT (bf16 out) with causal
    suffix trim, triangle masking on GPSIMD affine_select; per-head
    denominator reciprocal via the fast custom-DVE approx op.
"""

import numpy as np
import ml_dtypes

import concourse.bacc as bacc
import concourse.mybir as mybir
import concourse.tile as tile
from concourse.bass_utils import run_bass_kernel_spmd

F32 = mybir.dt.float32
BF16 = mybir.dt.bfloat16
EXP = mybir.ActivationFunctionType.Exp

B, S, D = 2, 2048, 1024
H, HD = 16, 64
THETA = 10000.0
NCORES = 8
NH = 4          # heads per core
C = NH * HD     # 256 channels per core
P = 128
DC = D // P     # 8 contraction chunks
NQT = S // 512  # 4 q-tiles
NTB = S // P    # 16 token blocks

_NC_CACHE = None
LAST_RESULTS = None


def _build():
    nc = bacc.Bacc(None)

    xT = nc.dram_tensor("xT", [D, S], BF16, kind="ExternalInput")
    wqT = nc.dram_tensor("wqT", [D, C], BF16, kind="ExternalInput")
    wkT = nc.dram_tensor("wkT", [D, C], BF16, kind="ExternalInput")
    wvT = nc.dram_tensor("wvT", [D, C], BF16, kind="ExternalInput")
    woT = nc.dram_tensor("woT", [C, D], BF16, kind="ExternalInput")
    cosP = nc.dram_tensor("cosP", [P, S], BF16, kind="ExternalInput")
    sinP = nc.dram_tensor("sinP", [P, S], BF16, kind="ExternalInput")
    out = nc.dram_tensor("out", [S, D], BF16, kind="ExternalOutput")

    xT3 = xT.rearrange("(dc di) t -> di dc t", di=P)
    wvT3 = wvT.rearrange("(dc di) c -> di dc c", di=P)
    woT3 = woT.rearrange("(cp ci) o -> ci cp o", ci=P)

    XOR1 = [i ^ 1 for i in range(32)]

    with tile.TileContext(nc) as tc:
        with (
            tc.tile_pool(name="cn", bufs=1) as cn,        # constants
            tc.tile_pool(name="big", bufs=1) as big,      # long-lived tensors
            tc.tile_pool(name="tmp", bufs=2) as tmp,      # rope scratch
            tc.tile_pool(name="ex", bufs=4) as ex,        # exp tiles
            tc.tile_pool(name="nrm", bufs=3) as nrm,      # normalize scratch
            tc.tile_pool(name="ob", bufs=3) as ob,        # out staging
            tc.tile_pool(name="psX", bufs=2, space="PSUM") as psX,  # 2 banks
        ):
            # ---- loads. Each dma_start costs ~650ns of HWDGE setup on
            # its issuing sequencer, so bulk loads go whole-tensor and
            # cos/sin/wo issue from the (idle) Scalar queue.
            wv_sb = cn.tile([P, DC, C], BF16, tag="wv")
            nc.sync.dma_start(wv_sb[:], wvT3[:])

            xt_sb = []
            for dc in range(DC):
                t = cn.tile([P, S], BF16, tag=f"xt{dc}", name=f"xt{dc}")
                xt_sb.append(t)
            # sync queue: first token chunks + wq/wk; scalar queue (idle
            # pre-phase): cos/sin, later token chunks, wo
            for dc in range(DC):
                nc.sync.dma_start(xt_sb[dc][:, 0:256], xT3[:, dc, 0:256])
            cos_sb = cn.tile([P, S], BF16, tag="cos")
            sin_sb = cn.tile([P, S], BF16, tag="sin")
            nc.scalar.dma_start(cos_sb[:], cosP[:])
            nc.scalar.dma_start(sin_sb[:], sinP[:])
            wq_sb = cn.tile([P, DC, C], BF16, tag="wq")
            wk_sb = cn.tile([P, DC, C], BF16, tag="wk")
            nc.sync.dma_start(
                wq_sb[:], wqT.rearrange("(dc di) c -> di dc c", di=P))
            nc.sync.dma_start(
                wk_sb[:], wkT.rearrange("(dc di) c -> di dc c", di=P))
            for dc in range(DC):
                nc.sync.dma_start(xt_sb[dc][:, 256:512], xT3[:, dc, 256:512])
            for dc in range(DC):
                nc.scalar.dma_start(
                    xt_sb[dc][:, 512:1024], xT3[:, dc, 512:1024])
            for dc in range(DC):
                nc.scalar.dma_start(
                    xt_sb[dc][:, 1024:2048], xT3[:, dc, 1024:2048])
            wo_sb = cn.tile([P, 2, D], BF16, tag="wo")
            nc.scalar.dma_start(wo_sb[:], woT3[:])

            ones_sb = cn.tile([P, NH], F32, tag="ones")
            nc.gpsimd.memset(ones_sb[:], 1.0)
            tri_sb = cn.tile([P, P], BF16, tag="tri")
            nc.gpsimd.memset(tri_sb[:], 1.0)
            nc.gpsimd.affine_select(
                tri_sb[:], tri_sb[:], [[1, P]], mybir.AluOpType.is_ge, 0.0,
                base=0, channel_multiplier=-1)

            vp_tiles = {}

            def emit_vpair(tb0):
                """V projection for token blocks tb0, tb0+1 (one PSUM bank)."""
                ps = psX.tile([P, 512], F32, tag="fx", name=f"v{tb0}")
                for j, tb in enumerate((tb0, tb0 + 1)):
                    cs = slice(256 * j, 256 * j + 256)
                    for dc in range(DC):
                        nc.tensor.matmul(
                            ps[:, cs], xt_sb[dc][:, tb * P:(tb + 1) * P],
                            wv_sb[:, dc, :],
                            start=(dc == 0), stop=(dc == DC - 1))
                for j, tb in enumerate((tb0, tb0 + 1)):
                    vp = big.tile([P, NH, 65], BF16, tag=f"vp{tb}",
                                  name=f"vp{tb}")
                    vp_tiles[tb] = vp
                    nc.vector.tensor_copy(vp[:, :, 64:65], ones_sb[:, :, None])
                    nc.vector.tensor_copy(
                        vp[:, :, 0:HD],
                        ps[:, 256 * j:256 * j + 256].rearrange(
                            "p (h c) -> p h c", c=HD))

            yt = {0: big.tile([P, S], BF16, tag="y0", name="y0"),
                  1: big.tile([P, S], BF16, tag="y1", name="y1")}

            def emit_outproj(tb, oc, tail=False):
                """Output projection for one (token block, 512-col half)."""
                po = psX.tile([P, 512], F32, tag="fx", name=f"o{tb}{oc}")
                tbs = slice(tb * P, (tb + 1) * P)
                for cp in range(2):
                    nc.tensor.matmul(
                        po[:], yt[cp][:, tbs],
                        wo_sb[:, cp, oc * 512:(oc + 1) * 512],
                        start=(cp == 0), stop=(cp == 1))
                ot = ob.tile([P, 512], BF16, tag="ot")
                if tail and oc == 1:
                    nc.scalar.copy(ot[:], po[:])
                else:
                    nc.vector.tensor_copy(ot[:], po[:])
                eng = nc.scalar if (tail and oc == 0) else nc.sync
                eng.dma_start(out[tbs, oc * 512:(oc + 1) * 512], ot[:])

            # ---- V projection for the first q-tile's k-blocks ----
            emit_vpair(0)
            emit_vpair(2)

            # ---- Q/K projections + rope (half-S PSUM tiles: 2 banks) ----
            qk_tiles = {}
            with tc.tile_pool(name="psQK", bufs=3, space="PSUM") as psQK:
                for proj, wT in (("q", wqT), ("k", wkT)):
                    for pair in range(2):
                        dst = big.tile([P, S], BF16, tag=f"{proj}{pair}",
                                       name=f"{proj}{pair}")
                        qk_tiles[(proj, pair)] = dst
                w_sbs = {"q": wq_sb, "k": wk_sb}
                for half in range(2):
                    hsl = slice(half * 1024, half * 1024 + 1024)
                    for proj in ("q", "k"):
                        for pair in range(2):
                            dst = qk_tiles[(proj, pair)]
                            ps = psQK.tile([P, 1024], F32, tag="qk")
                            for dc in range(DC):
                                w = w_sbs[proj][:, dc,
                                                pair * P:(pair + 1) * P]
                                for tt in range(2):
                                    csl = slice(half * 1024 + tt * 512,
                                                half * 1024 + tt * 512 + 512)
                                    nc.tensor.matmul(
                                        ps[:, tt * 512:(tt + 1) * 512],
                                        w, xt_sb[dc][:, csl],
                                        start=(dc == 0), stop=(dc == DC - 1))
                            sh = tmp.tile([P, 1024], F32, tag="sh")
                            t1 = tmp.tile([P, 1024], F32, tag="t1")
                            nc.vector.stream_shuffle(sh[:], ps[:], XOR1)
                            nc.vector.tensor_mul(t1[:], ps[:], cos_sb[:, hsl])
                            nc.gpsimd.tensor_mul(sh[:], sh[:], sin_sb[:, hsl])
                            nc.vector.tensor_add(
                                dst[:, half * 1024:half * 1024 + 512],
                                t1[:, 0:512], sh[:, 0:512])
                            nc.gpsimd.tensor_add(
                                dst[:, half * 1024 + 512:half * 1024 + 1024],
                                t1[:, 512:1024], sh[:, 512:1024])

            # ---- attention with interleaved filler PE work ----
            _attention(nc, tc, big, ex, nrm, qk_tiles, vp_tiles, yt,
                       emit_vpair, emit_outproj, tri_sb)

            # ---- last q-tile's output projection ----
            for tb in range(12, NTB):
                for oc in range(2):
                    emit_outproj(tb, oc, tail=True)

    nc.finalize()
    return nc


def _attention(nc, tc, big, ex, nrm, qk_tiles, vp_tiles, yt,
               emit_vpair, emit_outproj, tri_sb):
    """Causal attention, software-pipelined with PE filler work.

    Work items are (qt, pair, kblock-group) covering both heads; the
    two heads' score matmuls are emitted adjacently so they execute
    concurrently in distinct PE row groups (h0/h64). Each item's AV
    matmuls are deferred until after the NEXT item's scores + exp are
    emitted, so the strict-FIFO PE queue has score work while ACT runs
    the exp. V-projection and output-projection units are drizzled into
    the stream to keep the PE dense (HAM un-throttle).
    """
    with (
        tc.tile_pool(name="psSC", bufs=2, space="PSUM") as psSC,   # 4 banks
        tc.tile_pool(name="psAV", bufs=2, space="PSUM") as psAV,   # 2 banks
    ):
        avs = {}

        def geom(qt, grp):
            offs = [max(0, (kb - 4 * qt) * P) for kb in grp]
            ws = [512 - off for off in offs]
            slots = [0, 512][:len(grp)]
            return offs, ws, slots

        def emit_av(qt, pair, grp, last, ets):
            nkb = 4 * qt + 4
            offs, ws, slots = geom(qt, grp)
            av = avs[(qt, pair)]
            for o in range(2):
                h = 2 * pair + o
                for kb, off, w_, sl in zip(grp, offs, ws, slots):
                    nc.tensor.matmul(
                        av[o][:, off:512],
                        vp_tiles[kb][:, h, :],
                        ets[o][:, sl:sl + w_],
                        start=(kb == 0), stop=(kb == nkb - 1),
                        skip_group_check=True)
            if last:
                qs = slice(qt * 512, (qt + 1) * 512)
                for oo in range(2):
                    # custom-DVE reciprocal requires SBUF input; stage
                    # the PSUM denominator row first
                    din = nrm.tile([1, 512], F32, tag="din")
                    nc.vector.tensor_copy(din[:], av[oo][64:65, :])
                    rec = nrm.tile([1, 512], F32, tag="rec")
                    nc.vector.reciprocal_approx_fast(rec[:], din[:])
                    rb = nrm.tile([64, 512], F32, tag="rb")
                    nc.gpsimd.partition_broadcast(rb[:], rec[:])
                    nc.vector.tensor_mul(
                        yt[pair][64 * oo:64 * oo + 64, qs],
                        av[oo][0:64, :], rb[:])

        # absorb the psQK->psSC pool-transition stall (first score matmul
        # waits on the last QK-psum consumers) with V-proj work
        emit_vpair(4)
        emit_vpair(6)
        pend = None
        for qt in range(NQT):
            groups = [(kb, kb + 1) for kb in range(0, 4 * qt, 2)]
            groups.append((4 * qt, 4 * qt + 1))
            groups.append((4 * qt + 2,))
            groups.append((4 * qt + 3,))
            items = []
            for pair in range(2):
                for gi, grp in enumerate(groups):
                    items.append((qt, pair, grp, gi == 0,
                                  gi == len(groups) - 1))
            # filler units to interleave during this qt's attention:
            # V-proj for later q-tiles, out-proj of the PREVIOUS q-tile
            fillers = []
            if qt == 0:
                fillers.append(lambda: emit_vpair(8))
                fillers.append(lambda: emit_vpair(10))
            elif qt == 1:
                fillers.append(lambda: emit_vpair(12))
                fillers.append(lambda: emit_vpair(14))
            if qt >= 1:
                for tb in range(4 * (qt - 1), 4 * qt):
                    for oc in range(2):
                        fillers.append(
                            lambda tb=tb, oc=oc: emit_outproj(tb, oc))
            stride = max(1, len(items) // max(1, len(fillers)))

            for idx, (qt_, pair, grp, first, last) in enumerate(items):
                if first:
                    av = []
                    for oo in range(2):
                        avt = psAV.tile([P, 512], F32, tag="av",
                                        name=f"av{pair}{oo}")
                        av.append(avt[0:65])
                    avs[(qt_, pair)] = av
                qtile = qk_tiles[("q", pair)]
                ktile = qk_tiles[("k", pair)]
                offs, ws, slots = geom(qt_, grp)
                scs = []
                for o in range(2):
                    sct = psSC.tile([P, 1024], F32, tag="sc", name="sc")
                    scs.append(sct)
                # adjacent h0/h64 score matmuls run concurrently in
                # distinct PE row groups
                for kb, off, w_, sl in zip(grp, offs, ws, slots):
                    for o in range(2):
                        hs = slice(64 * o, 64 * o + 64)
                        nc.tensor.matmul(
                            scs[o][:, sl:sl + w_],
                            ktile[hs, kb * P:(kb + 1) * P],
                            qtile[hs, qt_ * 512 + off:(qt_ + 1) * 512],
                            start=True, stop=True)
                fd = slots[len(grp) - 1] + ws[len(grp) - 1]
                ets = []
                for o in range(2):
                    et = ex.tile([P, 1024], BF16, tag="e")
                    nc.scalar.activation(
                        et[:, 0:fd], scs[o][:, 0:fd], EXP, scale=0.125)
                    for kb, off, w_, sl in zip(grp, offs, ws, slots):
                        if kb >= 4 * qt_:
                            # causal triangle via DVE bf16 2x-mode multiply
                            nc.vector.tensor_mul(
                                et[:, sl:sl + P], et[:, sl:sl + P],
                                tri_sb[:])
                    ets.append(et)
                if pend is not None:
                    emit_av(*pend)
                pend = (qt_, pair, grp, last, ets)
                if fillers and idx % stride == stride - 1:
                    fillers.pop(0)()
            for f in fillers:
                f()
        emit_av(*pend)


def _prep_core_inputs(x, pos, Wq, Wk, Wv, Wo):
    """Per-core input dicts (host-side sharding + layout prep)."""
    bf16 = ml_dtypes.bfloat16
    inv_freq = THETA ** (-np.arange(0, HD, 2, dtype=np.float32) / HD)
    ang = pos.astype(np.float32)[:, None] * inv_freq[None, :]   # (S, 32)
    cos = np.cos(ang).astype(np.float32)                        # (S, 32)
    sin = np.sin(ang).astype(np.float32)
    p = np.arange(P)
    pairidx = (p % HD) // 2
    cosP = np.ascontiguousarray(cos[:, pairidx].T)              # (128, S)
    sgn = np.where(p % 2 == 0, -1.0, 1.0).astype(np.float32)
    sinP = np.ascontiguousarray(sin[:, pairidx].T * sgn[:, None])

    xTs = [np.ascontiguousarray(x[b].T).astype(bf16) for b in range(B)]
    maps = []
    for c in range(NCORES):
        b, g = divmod(c, NH)
        cs = slice(C * g, C * (g + 1))
        maps.append({
            "xT": xTs[b],
            "wqT": np.ascontiguousarray(Wq[cs, :].T).astype(bf16),
            "wkT": np.ascontiguousarray(Wk[cs, :].T).astype(bf16),
            "wvT": np.ascontiguousarray(Wv[cs, :].T).astype(bf16),
            "woT": np.ascontiguousarray(Wo[:, cs].T).astype(bf16),
            "cosP": cosP.astype(bf16),
            "sinP": sinP.astype(bf16),
        })
    return maps


def kernel(in_features, token_positions, Wq, Wk, Wv, Wo):
    global _NC_CACHE, LAST_RESULTS
    x = np.asarray(in_features, dtype=np.float32)
    pos = np.asarray(token_positions)
    Wq = np.asarray(Wq, dtype=np.float32)
    Wk = np.asarray(Wk, dtype=np.float32)
    Wv = np.asarray(Wv, dtype=np.float32)
    Wo = np.asarray(Wo, dtype=np.float32)

    if _NC_CACHE is None:
        _NC_CACHE = _build()
    maps = _prep_core_inputs(x, pos, Wq, Wk, Wv, Wo)
    res = run_bass_kernel_spmd(_NC_CACHE, maps, core_ids=list(range(NCORES)))
    LAST_RESULTS = res
    parts = [np.asarray(r["out"], dtype=np.float32) for r in res.results]
    outb = [parts[4 * b] + parts[4 * b + 1] + parts[4 * b + 2] + parts[4 * b + 3]
            for b in range(B)]
    return np.stack(outb).astype(np.float32)


if __name__ == "__main__":
    rng = np.random.default_rng(0)
    x = rng.standard_normal((B, S, D), dtype=np.float32)
    o = kernel(x, np.arange(S, dtype=np.int32),
               *(rng.standard_normal((D, D), dtype=np.float32) / 32
                 for _ in range(4)))
    print(o.shape, o.dtype)


# revision 30
# speedup vs baseline: 1.0036x; 1.0036x over previous
"""MultiHeadAttention with RoPE on 8 Trainium2 NeuronCores.

Sharding: batch (2) x head-group (4 heads each) -> 8 cores. Each core
computes q/k/v projections for its 4 heads of one batch element, causal
attention, and a partial output projection (row-shard of Wo). The host
sums the 4 partial outputs per batch element (the "all-reduce").

All matmul operands are bf16 (PSUM accumulates fp32): fp32 HIGH-mode
matmuls disable FWL and run multi-pass; bf16 streams 1 col/cycle.

The attention phase is rate-limited by ACT (exp ~864ns/item vs ~430ns
of PE score+AV work per item), which starves the PE and lets the HAM
clock gate throttle it to 1.2 GHz. Countermeasures:
  - AV matmuls are deferred one work item behind their exp (software
    pipeline) so the PE always has score work queued while ACT runs.
  - The V projection for token blocks 4..15 and the per-q-tile output
    projection are interleaved into the attention stream as PE filler.
PSUM budget: psSC 2x[128,1024] (4 banks) + psAV 2x[128,512] (2) +
psX filler 2x[128,512] (2) = 8 banks.

Device layout per core:
  - x.T (d-major, bf16) streamed to SBUF once (token-major chunks so
    compute starts early); all projections contract over d.
  - Q/K produced channel-partition (Q.T layout); RoPE applied via DVE
    stream_shuffle (partition XOR-1) + cos/sin tables, result cast bf16.
  - scores computed transposed (k on partitions, q on free) so the AV
    matmul needs no transposes; the two heads of a pair go to partition
    bases 0/64 (row groups h0/h64).
  - softmax denominators come free from an extra ones-column in the
    V-stationary AV matmul (M=65); exp on ACT (bf16 out) with causal
    suffix trim, triangle masking on GPSIMD affine_select; per-head
    denominator reciprocal via the fast custom-DVE approx op.
"""

import numpy as np
import ml_dtypes

import concourse.bacc as bacc
import concourse.mybir as mybir
import concourse.tile as tile
from concourse.bass_utils import run_bass_kernel_spmd

F32 = mybir.dt.float32
BF16 = mybir.dt.bfloat16
EXP = mybir.ActivationFunctionType.Exp

B, S, D = 2, 2048, 1024
H, HD = 16, 64
THETA = 10000.0
NCORES = 8
NH = 4          # heads per core
C = NH * HD     # 256 channels per core
P = 128
DC = D // P     # 8 contraction chunks
NQT = S // 512  # 4 q-tiles
NTB = S // P    # 16 token blocks

_NC_CACHE = None
LAST_RESULTS = None


def _build():
    nc = bacc.Bacc(None)

    xT = nc.dram_tensor("xT", [D, S], BF16, kind="ExternalInput")
    wqT = nc.dram_tensor("wqT", [D, C], BF16, kind="ExternalInput")
    wkT = nc.dram_tensor("wkT", [D, C], BF16, kind="ExternalInput")
    wvT = nc.dram_tensor("wvT", [D, C], BF16, kind="ExternalInput")
    woT = nc.dram_tensor("woT", [C, D], BF16, kind="ExternalInput")
    cosP = nc.dram_tensor("cosP", [P, S], BF16, kind="ExternalInput")
    sinP = nc.dram_tensor("sinP", [P, S], BF16, kind="ExternalInput")
    out = nc.dram_tensor("out", [S, D], BF16, kind="ExternalOutput")

    xT3 = xT.rearrange("(dc di) t -> di dc t", di=P)
    wvT3 = wvT.rearrange("(dc di) c -> di dc c", di=P)
    woT3 = woT.rearrange("(cp ci) o -> ci cp o", ci=P)

    XOR1 = [i ^ 1 for i in range(32)]

    with tile.TileContext(nc) as tc:
        with (
            tc.tile_pool(name="cn", bufs=1) as cn,        # constants
            tc.tile_pool(name="big", bufs=1) as big,      # long-lived tensors
            tc.tile_pool(name="tmp", bufs=2) as tmp,      # rope scratch
            tc.tile_pool(name="ex", bufs=3) as ex,        # exp tiles
            tc.tile_pool(name="nrm", bufs=3) as nrm,      # normalize scratch
            tc.tile_pool(name="ob", bufs=3) as ob,        # out staging
            tc.tile_pool(name="psX", bufs=2, space="PSUM") as psX,  # 2 banks
        ):
            # ---- loads. Each dma_start costs ~650ns of HWDGE setup on
            # its issuing sequencer, so bulk loads go whole-tensor and
            # cos/sin/wo issue from the (idle) Scalar queue.
            wv_sb = cn.tile([P, DC, C], BF16, tag="wv")
            nc.sync.dma_start(wv_sb[:], wvT3[:])

            xt_sb = []
            for dc in range(DC):
                t = cn.tile([P, S], BF16, tag=f"xt{dc}", name=f"xt{dc}")
                xt_sb.append(t)
            # sync queue: first token chunks + wq/wk; scalar queue (idle
            # pre-phase): cos/sin, later token chunks, wo
            for dc in range(DC):
                nc.sync.dma_start(xt_sb[dc][:, 0:256], xT3[:, dc, 0:256])
            cos_sb = cn.tile([P, S], BF16, tag="cos")
            sin_sb = cn.tile([P, S], BF16, tag="sin")
            nc.scalar.dma_start(cos_sb[:], cosP[:])
            nc.scalar.dma_start(sin_sb[:], sinP[:])
            for dc in range(DC):
                nc.sync.dma_start(xt_sb[dc][:, 256:512], xT3[:, dc, 256:512])
            for dc in range(DC):
                nc.scalar.dma_start(
                    xt_sb[dc][:, 512:1024], xT3[:, dc, 512:1024])
            wq_sb = cn.tile([P, DC, C], BF16, tag="wq")
            wk_sb = cn.tile([P, DC, C], BF16, tag="wk")
            nc.sync.dma_start(
                wq_sb[:], wqT.rearrange("(dc di) c -> di dc c", di=P))
            nc.sync.dma_start(
                wk_sb[:], wkT.rearrange("(dc di) c -> di dc c", di=P))
            for dc in range(DC):
                nc.scalar.dma_start(
                    xt_sb[dc][:, 1024:2048], xT3[:, dc, 1024:2048])
            wo_sb = cn.tile([P, 2, D], BF16, tag="wo")
            nc.scalar.dma_start(wo_sb[:], woT3[:])

            ones_sb = cn.tile([P, NH], F32, tag="ones")
            nc.gpsimd.memset(ones_sb[:], 1.0)
            tri_sb = cn.tile([P, P], BF16, tag="tri")
            nc.gpsimd.memset(tri_sb[:], 1.0)
            nc.gpsimd.affine_select(
                tri_sb[:], tri_sb[:], [[1, P]], mybir.AluOpType.is_ge, 0.0,
                base=0, channel_multiplier=-1)

            vp_tiles = {}

            def emit_vpair(tb0):
                """V projection for token blocks tb0, tb0+1 (one PSUM bank)."""
                ps = psX.tile([P, 512], F32, tag="fx", name=f"v{tb0}")
                for j, tb in enumerate((tb0, tb0 + 1)):
                    cs = slice(256 * j, 256 * j + 256)
                    for dc in range(DC):
                        nc.tensor.matmul(
                            ps[:, cs], xt_sb[dc][:, tb * P:(tb + 1) * P],
                            wv_sb[:, dc, :],
                            start=(dc == 0), stop=(dc == DC - 1))
                for j, tb in enumerate((tb0, tb0 + 1)):
                    vp = big.tile([P, NH, 65], BF16, tag=f"vp{tb}",
                                  name=f"vp{tb}")
                    vp_tiles[tb] = vp
                    nc.vector.tensor_copy(vp[:, :, 64:65], ones_sb[:, :, None])
                    nc.vector.tensor_copy(
                        vp[:, :, 0:HD],
                        ps[:, 256 * j:256 * j + 256].rearrange(
                            "p (h c) -> p h c", c=HD))

            yt = {0: big.tile([P, S], BF16, tag="y0", name="y0"),
                  1: big.tile([P, S], BF16, tag="y1", name="y1")}

            def emit_outproj(tb, oc, tail=False):
                """Output projection for one (token block, 512-col half)."""
                po = psX.tile([P, 512], F32, tag="fx", name=f"o{tb}{oc}")
                tbs = slice(tb * P, (tb + 1) * P)
                for cp in range(2):
                    nc.tensor.matmul(
                        po[:], yt[cp][:, tbs],
                        wo_sb[:, cp, oc * 512:(oc + 1) * 512],
                        start=(cp == 0), stop=(cp == 1))
                ot = ob.tile([P, 512], BF16, tag="ot")
                if tail and oc == 1:
                    nc.scalar.copy(ot[:], po[:])
                else:
                    nc.vector.tensor_copy(ot[:], po[:])
                eng = nc.scalar if (tail and oc == 0) else nc.sync
                eng.dma_start(out[tbs, oc * 512:(oc + 1) * 512], ot[:])

            # ---- V projection for the first q-tile's k-blocks ----
            emit_vpair(0)
            emit_vpair(2)

            # ---- Q/K projections + rope (half-S PSUM tiles: 2 banks) ----
            qk_tiles = {}
            with tc.tile_pool(name="psQK", bufs=3, space="PSUM") as psQK:
                for proj, wT in (("q", wqT), ("k", wkT)):
                    for pair in range(2):
                        dst = big.tile([P, S], BF16, tag=f"{proj}{pair}",
                                       name=f"{proj}{pair}")
                        qk_tiles[(proj, pair)] = dst
                w_sbs = {"q": wq_sb, "k": wk_sb}
                for half in range(2):
                    hsl = slice(half * 1024, half * 1024 + 1024)
                    for proj in ("q", "k"):
                        for pair in range(2):
                            dst = qk_tiles[(proj, pair)]
                            ps = psQK.tile([P, 1024], F32, tag="qk")
                            for dc in range(DC):
                                w = w_sbs[proj][:, dc,
                                                pair * P:(pair + 1) * P]
                                for tt in range(2):
                                    csl = slice(half * 1024 + tt * 512,
                                                half * 1024 + tt * 512 + 512)
                                    nc.tensor.matmul(
                                        ps[:, tt * 512:(tt + 1) * 512],
                                        w, xt_sb[dc][:, csl],
                                        start=(dc == 0), stop=(dc == DC - 1))
                            sh = tmp.tile([P, 1024], F32, tag="sh")
                            t1 = tmp.tile([P, 1024], F32, tag="t1")
                            nc.vector.stream_shuffle(sh[:], ps[:], XOR1)
                            nc.vector.tensor_mul(t1[:], ps[:], cos_sb[:, hsl])
                            nc.gpsimd.tensor_mul(sh[:], sh[:], sin_sb[:, hsl])
                            nc.vector.tensor_add(
                                dst[:, half * 1024:half * 1024 + 512],
                                t1[:, 0:512], sh[:, 0:512])
                            nc.gpsimd.tensor_add(
                                dst[:, half * 1024 + 512:half * 1024 + 1024],
                                t1[:, 512:1024], sh[:, 512:1024])

            # ---- attention with interleaved filler PE work ----
            _attention(nc, tc, big, ex, nrm, qk_tiles, vp_tiles, yt,
                       emit_vpair, emit_outproj, tri_sb)

            # ---- last q-tile's output projection ----
            for tb in range(12, NTB):
                for oc in range(2):
                    emit_outproj(tb, oc, tail=True)

    nc.finalize()
    return nc


def _attention(nc, tc, big, ex, nrm, qk_tiles, vp_tiles, yt,
               emit_vpair, emit_outproj, tri_sb):
    """Causal attention, software-pipelined with PE filler work.

    Work items are (qt, pair, kblock-group) covering both heads; the
    two heads' score matmuls are emitted adjacently so they execute
    concurrently in distinct PE row groups (h0/h64). Each item's AV
    matmuls are deferred until after the NEXT item's scores + exp are
    emitted, so the strict-FIFO PE queue has score work while ACT runs
    the exp. V-projection and output-projection units are drizzled into
    the stream to keep the PE dense (HAM un-throttle).
    """
    with (
        tc.tile_pool(name="psSC", bufs=2, space="PSUM") as psSC,   # 4 banks
        tc.tile_pool(name="psAV", bufs=2, space="PSUM") as psAV,   # 2 banks
    ):
        avs = {}

        def geom(qt, grp):
            offs = [max(0, (kb - 4 * qt) * P) for kb in grp]
            ws = [512 - off for off in offs]
            slots = [0, 512][:len(grp)]
            return offs, ws, slots

        def emit_av(qt, pair, grp, o, last, et):
            nkb = 4 * qt + 4
            offs, ws, slots = geom(qt, grp)
            av = avs[(qt, pair)]
            h = 2 * pair + o
            for kb, off, w_, sl in zip(grp, offs, ws, slots):
                nc.tensor.matmul(
                    av[o][:, off:512],
                    vp_tiles[kb][:, h, :],
                    et[:, sl:sl + w_],
                    start=(kb == 0), stop=(kb == nkb - 1),
                    skip_group_check=True)
            if last and o == 1:
                qs = slice(qt * 512, (qt + 1) * 512)
                for oo in range(2):
                    # custom-DVE reciprocal requires SBUF input; stage
                    # the PSUM denominator row first
                    din = nrm.tile([1, 512], F32, tag="din")
                    nc.vector.tensor_copy(din[:], av[oo][64:65, :])
                    rec = nrm.tile([1, 512], F32, tag="rec")
                    nc.vector.reciprocal_approx_fast(rec[:], din[:])
                    rb = nrm.tile([64, 512], F32, tag="rb")
                    nc.gpsimd.partition_broadcast(rb[:], rec[:])
                    nc.vector.tensor_mul(
                        yt[pair][64 * oo:64 * oo + 64, qs],
                        av[oo][0:64, :], rb[:])

        # absorb the psQK->psSC pool-transition stall (first score matmul
        # waits on the last QK-psum consumers) with V-proj work
        emit_vpair(4)
        emit_vpair(6)
        pend = None
        for qt in range(NQT):
            groups = [(kb, kb + 1) for kb in range(0, 4 * qt, 2)]
            groups.append((4 * qt, 4 * qt + 1))
            groups.append((4 * qt + 2,))
            groups.append((4 * qt + 3,))
            items = []
            for pair in range(2):
                for gi, grp in enumerate(groups):
                    for o in range(2):
                        items.append((qt, pair, grp, o, gi == 0,
                                      gi == len(groups) - 1))
            # filler units to interleave during this qt's attention:
            # V-proj for later q-tiles, out-proj of the PREVIOUS q-tile
            fillers = []
            if qt == 0:
                fillers.append(lambda: emit_vpair(8))
                fillers.append(lambda: emit_vpair(10))
            elif qt == 1:
                fillers.append(lambda: emit_vpair(12))
                fillers.append(lambda: emit_vpair(14))
            if qt >= 1:
                for tb in range(4 * (qt - 1), 4 * qt):
                    for oc in range(2):
                        fillers.append(
                            lambda tb=tb, oc=oc: emit_outproj(tb, oc))
            stride = max(1, len(items) // max(1, len(fillers)))

            for idx, (qt_, pair, grp, o, first, last) in enumerate(items):
                if first and o == 0:
                    av = []
                    for oo in range(2):
                        avt = psAV.tile([P, 512], F32, tag="av",
                                        name=f"av{pair}{oo}")
                        av.append(avt[0:65])
                    avs[(qt_, pair)] = av
                qtile = qk_tiles[("q", pair)]
                ktile = qk_tiles[("k", pair)]
                offs, ws, slots = geom(qt_, grp)
                hs = slice(64 * o, 64 * o + 64)
                sct = psSC.tile([P, 1024], F32, tag="sc", name="sc")
                for kb, off, w_, sl in zip(grp, offs, ws, slots):
                    nc.tensor.matmul(
                        sct[:, sl:sl + w_],
                        ktile[hs, kb * P:(kb + 1) * P],
                        qtile[hs, qt_ * 512 + off:(qt_ + 1) * 512],
                        start=True, stop=True)
                fd = slots[len(grp) - 1] + ws[len(grp) - 1]
                et = ex.tile([P, 1024], BF16, tag="e")
                nc.scalar.activation(
                    et[:, 0:fd], sct[:, 0:fd], EXP, scale=0.125)
                for kb, off, w_, sl in zip(grp, offs, ws, slots):
                    if kb >= 4 * qt_:
                        # causal triangle via DVE bf16 2x-mode multiply
                        nc.vector.tensor_mul(
                            et[:, sl:sl + P], et[:, sl:sl + P], tri_sb[:])
                if pend is not None:
                    emit_av(*pend)
                pend = (qt_, pair, grp, o, last, et)
                if fillers and idx % stride == stride - 1:
                    fillers.pop(0)()
            for f in fillers:
                f()
        emit_av(*pend)


def _prep_core_inputs(x, pos, Wq, Wk, Wv, Wo):
    """Per-core input dicts (host-side sharding + layout prep)."""
    bf16 = ml_dtypes.bfloat16
    inv_freq = THETA ** (-np.arange(0, HD, 2, dtype=np.float32) / HD)
    ang = pos.astype(np.float32)[:, None] * inv_freq[None, :]   # (S, 32)
    cos = np.cos(ang).astype(np.float32)                        # (S, 32)
    sin = np.sin(ang).astype(np.float32)
    p = np.arange(P)
    pairidx = (p % HD) // 2
    cosP = np.ascontiguousarray(cos[:, pairidx].T)              # (128, S)
    sgn = np.where(p % 2 == 0, -1.0, 1.0).astype(np.float32)
    sinP = np.ascontiguousarray(sin[:, pairidx].T * sgn[:, None])

    xTs = [np.ascontiguousarray(x[b].T).astype(bf16) for b in range(B)]
    maps = []
    for c in range(NCORES):
        b, g = divmod(c, NH)
        cs = slice(C * g, C * (g + 1))
        maps.append({
            "xT": xTs[b],
            "wqT": np.ascontiguousarray(Wq[cs, :].T).astype(bf16),
            "wkT": np.ascontiguousarray(Wk[cs, :].T).astype(bf16),
            "wvT": np.ascontiguousarray(Wv[cs, :].T).astype(bf16),
            "woT": np.ascontiguousarray(Wo[:, cs].T).astype(bf16),
            "cosP": cosP.astype(bf16),
            "sinP": sinP.astype(bf16),
        })
    return maps


def kernel(in_features, token_positions, Wq, Wk, Wv, Wo):
    global _NC_CACHE, LAST_RESULTS
    x = np.asarray(in_features, dtype=np.float32)
    pos = np.asarray(token_positions)
    Wq = np.asarray(Wq, dtype=np.float32)
    Wk = np.asarray(Wk, dtype=np.float32)
    Wv = np.asarray(Wv, dtype=np.float32)
    Wo = np.asarray(Wo, dtype=np.float32)

    if _NC_CACHE is None:
        _NC_CACHE = _build()
    maps = _prep_core_inputs(x, pos, Wq, Wk, Wv, Wo)
    res = run_bass_kernel_spmd(_NC_CACHE, maps, core_ids=list(range(NCORES)))
    LAST_RESULTS = res
    parts = [np.asarray(r["out"], dtype=np.float32) for r in res.results]
    outb = [parts[4 * b] + parts[4 * b + 1] + parts[4 * b + 2] + parts[4 * b + 3]
            for b in range(B)]
    return np.stack(outb).astype(np.float32)


if __name__ == "__main__":
    rng = np.random.default_rng(0)
    x = rng.standard_normal((B, S, D), dtype=np.float32)
    o = kernel(x, np.arange(S, dtype=np.int32),
               *(rng.standard_normal((D, D), dtype=np.float32) / 32
                 for _ in range(4)))
    print(o.shape, o.dtype)


# revision 32
# speedup vs baseline: 1.0153x; 1.0116x over previous
"""MultiHeadAttention with RoPE on 8 Trainium2 NeuronCores.

Sharding: batch (2) x head-group (4 heads each) -> 8 cores. Each core
computes q/k/v projections for its 4 heads of one batch element, causal
attention, and a partial output projection (row-shard of Wo). The host
sums the 4 partial outputs per batch element (the "all-reduce").

All matmul operands are bf16 (PSUM accumulates fp32): fp32 HIGH-mode
matmuls disable FWL and run multi-pass; bf16 streams 1 col/cycle.

The attention phase is rate-limited by ACT (exp ~864ns/item vs ~430ns
of PE score+AV work per item), which starves the PE and lets the HAM
clock gate throttle it to 1.2 GHz. Countermeasures:
  - AV matmuls are deferred one work item behind their exp (software
    pipeline) so the PE always has score work queued while ACT runs.
  - The V projection for token blocks 4..15 and the per-q-tile output
    projection are interleaved into the attention stream as PE filler.
PSUM budget: psSC 2x[128,1024] (4 banks) + psAV 2x[128,512] (2) +
psX filler 2x[128,512] (2) = 8 banks.

Device layout per core:
  - x.T (d-major, bf16) streamed to SBUF once (token-major chunks so
    compute starts early); all projections contract over d.
  - Q/K produced channel-partition (Q.T layout); RoPE applied via DVE
    stream_shuffle (partition XOR-1) + cos/sin tables, result cast bf16.
  - scores computed transposed (k on partitions, q on free) so the AV
    matmul needs no transposes; the two heads of a pair go to partition
    bases 0/64 (row groups h0/h64).
  - softmax denominators come free from an extra ones-column in the
    V-stationary AV matmul (M=65); exp on ACT (bf16 out) with causal
    suffix trim, triangle masking via a DVE bf16 multiply with a
    precomputed triangle tile; per-head denominator reciprocal via the
    fast custom-DVE approx op; bf16 partial outputs summed on the host.
"""

import numpy as np
import ml_dtypes

import concourse.bacc as bacc
import concourse.mybir as mybir
import concourse.tile as tile
from concourse.bass_utils import run_bass_kernel_spmd

F32 = mybir.dt.float32
BF16 = mybir.dt.bfloat16
EXP = mybir.ActivationFunctionType.Exp

B, S, D = 2, 2048, 1024
H, HD = 16, 64
THETA = 10000.0
NCORES = 8
NH = 4          # heads per core
C = NH * HD     # 256 channels per core
P = 128
DC = D // P     # 8 contraction chunks
NQT = S // 512  # 4 q-tiles
NTB = S // P    # 16 token blocks

_NC_CACHE = None
LAST_RESULTS = None


def _build():
    nc = bacc.Bacc(None)

    xT = nc.dram_tensor("xT", [D, S], BF16, kind="ExternalInput")
    wqT = nc.dram_tensor("wqT", [D, C], BF16, kind="ExternalInput")
    wkT = nc.dram_tensor("wkT", [D, C], BF16, kind="ExternalInput")
    wvT = nc.dram_tensor("wvT", [D, C], BF16, kind="ExternalInput")
    woT = nc.dram_tensor("woT", [C, D], BF16, kind="ExternalInput")
    cosP = nc.dram_tensor("cosP", [P, S], BF16, kind="ExternalInput")
    sinP = nc.dram_tensor("sinP", [P, S], BF16, kind="ExternalInput")
    out = nc.dram_tensor("out", [S, D], BF16, kind="ExternalOutput")

    xT3 = xT.rearrange("(dc di) t -> di dc t", di=P)
    wvT3 = wvT.rearrange("(dc di) c -> di dc c", di=P)
    woT3 = woT.rearrange("(cp ci) o -> ci cp o", ci=P)

    XOR1 = [i ^ 1 for i in range(32)]

    with tile.TileContext(nc) as tc:
        with (
            tc.tile_pool(name="cn", bufs=1) as cn,        # constants
            tc.tile_pool(name="big", bufs=1) as big,      # long-lived tensors
            tc.tile_pool(name="tmp", bufs=2) as tmp,      # rope scratch
            tc.tile_pool(name="ex", bufs=3) as ex,        # exp tiles
            tc.tile_pool(name="nrm", bufs=3) as nrm,      # normalize scratch
            tc.tile_pool(name="ob", bufs=3) as ob,        # out staging
            tc.tile_pool(name="psX", bufs=1, space="PSUM") as psX,  # 1 bank
        ):
            # ---- loads. Each dma_start costs ~650ns of HWDGE setup on
            # its issuing sequencer, so bulk loads go whole-tensor and
            # cos/sin/wo issue from the (idle) Scalar queue.
            wv_sb = cn.tile([P, DC, C], BF16, tag="wv")
            nc.sync.dma_start(wv_sb[:], wvT3[:])

            # one [P, DC, S] tile: each token chunk is a single strided
            # DMA (vs 8) -- the ~650ns/issue HWDGE setup was serializing
            # the sync sequencer. sync: x-chunks + wq/wk; scalar: cos/sin
            # + later chunks + wo.
            xt3_sb = cn.tile([P, DC, S], BF16, tag="xt")
            xt_sb = [xt3_sb[:, dc, :] for dc in range(DC)]
            nc.sync.dma_start(xt3_sb[:, :, 0:256], xT3[:, :, 0:256])
            cos_sb = cn.tile([P, S], BF16, tag="cos")
            sin_sb = cn.tile([P, S], BF16, tag="sin")
            nc.scalar.dma_start(cos_sb[:], cosP[:])
            nc.scalar.dma_start(sin_sb[:], sinP[:])
            nc.sync.dma_start(xt3_sb[:, :, 256:512], xT3[:, :, 256:512])
            nc.scalar.dma_start(xt3_sb[:, :, 512:1024], xT3[:, :, 512:1024])
            wq_sb = cn.tile([P, DC, C], BF16, tag="wq")
            wk_sb = cn.tile([P, DC, C], BF16, tag="wk")
            nc.sync.dma_start(
                wq_sb[:], wqT.rearrange("(dc di) c -> di dc c", di=P))
            nc.sync.dma_start(
                wk_sb[:], wkT.rearrange("(dc di) c -> di dc c", di=P))
            nc.scalar.dma_start(xt3_sb[:, :, 1024:2048], xT3[:, :, 1024:2048])
            wo_sb = cn.tile([P, 2, D], BF16, tag="wo")
            nc.scalar.dma_start(wo_sb[:], woT3[:])

            ones_sb = cn.tile([P, NH], F32, tag="ones")
            nc.gpsimd.memset(ones_sb[:], 1.0)
            tri_sb = cn.tile([P, P], BF16, tag="tri")
            nc.gpsimd.memset(tri_sb[:], 1.0)
            nc.gpsimd.affine_select(
                tri_sb[:], tri_sb[:], [[1, P]], mybir.AluOpType.is_ge, 0.0,
                base=0, channel_multiplier=-1)

            vp_tiles = {}

            def emit_vpair(tb0):
                """V projection for token blocks tb0, tb0+1 (one PSUM bank)."""
                ps = psX.tile([P, 512], F32, tag="fx", name=f"v{tb0}")
                for j, tb in enumerate((tb0, tb0 + 1)):
                    cs = slice(256 * j, 256 * j + 256)
                    for dc in range(DC):
                        nc.tensor.matmul(
                            ps[:, cs], xt_sb[dc][:, tb * P:(tb + 1) * P],
                            wv_sb[:, dc, :],
                            start=(dc == 0), stop=(dc == DC - 1))
                for j, tb in enumerate((tb0, tb0 + 1)):
                    vp = big.tile([P, NH, 65], BF16, tag=f"vp{tb}",
                                  name=f"vp{tb}")
                    vp_tiles[tb] = vp
                    nc.vector.tensor_copy(vp[:, :, 64:65], ones_sb[:, :, None])
                    nc.vector.tensor_copy(
                        vp[:, :, 0:HD],
                        ps[:, 256 * j:256 * j + 256].rearrange(
                            "p (h c) -> p h c", c=HD))

            yt = {0: big.tile([P, S], BF16, tag="y0", name="y0"),
                  1: big.tile([P, S], BF16, tag="y1", name="y1")}

            def emit_outproj(tb, oc, tail=False):
                """Output projection for one (token block, 512-col half)."""
                po = psX.tile([P, 512], F32, tag="fx", name=f"o{tb}{oc}")
                tbs = slice(tb * P, (tb + 1) * P)
                for cp in range(2):
                    nc.tensor.matmul(
                        po[:], yt[cp][:, tbs],
                        wo_sb[:, cp, oc * 512:(oc + 1) * 512],
                        start=(cp == 0), stop=(cp == 1))
                ot = ob.tile([P, 512], BF16, tag="ot")
                if tail and oc == 1:
                    nc.scalar.copy(ot[:], po[:])
                else:
                    nc.vector.tensor_copy(ot[:], po[:])
                eng = nc.scalar if (tail and oc == 0) else nc.sync
                eng.dma_start(out[tbs, oc * 512:(oc + 1) * 512], ot[:])

            # ---- V projection for the first q-tile's k-blocks ----
            emit_vpair(0)
            emit_vpair(2)

            # ---- Q/K projections + rope (half-S PSUM tiles: 2 banks) ----
            qk_tiles = {}
            with tc.tile_pool(name="psQK", bufs=3, space="PSUM") as psQK:
                for proj, wT in (("q", wqT), ("k", wkT)):
                    for pair in range(2):
                        dst = big.tile([P, S], BF16, tag=f"{proj}{pair}",
                                       name=f"{proj}{pair}")
                        qk_tiles[(proj, pair)] = dst
                w_sbs = {"q": wq_sb, "k": wk_sb}
                for half in range(2):
                    hsl = slice(half * 1024, half * 1024 + 1024)
                    for proj in ("q", "k"):
                        for pair in range(2):
                            dst = qk_tiles[(proj, pair)]
                            ps = psQK.tile([P, 1024], F32, tag="qk")
                            for dc in range(DC):
                                w = w_sbs[proj][:, dc,
                                                pair * P:(pair + 1) * P]
                                for tt in range(2):
                                    csl = slice(half * 1024 + tt * 512,
                                                half * 1024 + tt * 512 + 512)
                                    nc.tensor.matmul(
                                        ps[:, tt * 512:(tt + 1) * 512],
                                        w, xt_sb[dc][:, csl],
                                        start=(dc == 0), stop=(dc == DC - 1))
                            sh = tmp.tile([P, 1024], F32, tag="sh")
                            t1 = tmp.tile([P, 1024], F32, tag="t1")
                            nc.vector.stream_shuffle(sh[:], ps[:], XOR1)
                            nc.vector.tensor_mul(t1[:], ps[:], cos_sb[:, hsl])
                            nc.gpsimd.tensor_mul(sh[:], sh[:], sin_sb[:, hsl])
                            nc.vector.tensor_add(
                                dst[:, half * 1024:half * 1024 + 512],
                                t1[:, 0:512], sh[:, 0:512])
                            nc.gpsimd.tensor_add(
                                dst[:, half * 1024 + 512:half * 1024 + 1024],
                                t1[:, 512:1024], sh[:, 512:1024])

            # ---- attention with interleaved filler PE work ----
            _attention(nc, tc, big, ex, nrm, qk_tiles, vp_tiles, yt,
                       emit_vpair, emit_outproj, tri_sb)

            # ---- last q-tile's output projection ----
            for tb in range(12, NTB):
                for oc in range(2):
                    emit_outproj(tb, oc, tail=True)

    nc.finalize()
    return nc


def _attention(nc, tc, big, ex, nrm, qk_tiles, vp_tiles, yt,
               emit_vpair, emit_outproj, tri_sb):
    """Causal attention, software-pipelined with PE filler work.

    Work items are (qt, pair, kblock-group) covering both heads; the
    two heads' score matmuls are emitted adjacently so they execute
    concurrently in distinct PE row groups (h0/h64). Each item's AV
    matmuls are deferred until after the NEXT item's scores + exp are
    emitted, so the strict-FIFO PE queue has score work while ACT runs
    the exp. V-projection and output-projection units are drizzled into
    the stream to keep the PE dense (HAM un-throttle).
    """
    with (
        tc.tile_pool(name="psSC", bufs=2, space="PSUM") as psSC,   # 4 banks
        tc.tile_pool(name="psAV", bufs=3, space="PSUM") as psAV,   # 3 banks
    ):
        avs = {}

        def geom(qt, grp):
            offs = [max(0, (kb - 4 * qt) * P) for kb in grp]
            ws = [512 - off for off in offs]
            slots = [0, 512][:len(grp)]
            return offs, ws, slots

        def emit_av(qt, pair, grp, o, last, et):
            nkb = 4 * qt + 4
            offs, ws, slots = geom(qt, grp)
            av = avs[(qt, pair)]
            h = 2 * pair + o
            for kb, off, w_, sl in zip(grp, offs, ws, slots):
                nc.tensor.matmul(
                    av[o][:, off:512],
                    vp_tiles[kb][:, h, :],
                    et[:, sl:sl + w_],
                    start=(kb == 0), stop=(kb == nkb - 1),
                    skip_group_check=True)
            if last and o == 1:
                qs = slice(qt * 512, (qt + 1) * 512)
                for oo in range(2):
                    # custom-DVE reciprocal requires SBUF input; stage
                    # the PSUM denominator row first
                    din = nrm.tile([1, 512], F32, tag="din")
                    nc.vector.tensor_copy(din[:], av[oo][64:65, :])
                    rec = nrm.tile([1, 512], F32, tag="rec")
                    nc.vector.reciprocal_approx_fast(rec[:], din[:])
                    rb = nrm.tile([64, 512], F32, tag="rb")
                    nc.gpsimd.partition_broadcast(rb[:], rec[:])
                    nc.vector.tensor_mul(
                        yt[pair][64 * oo:64 * oo + 64, qs],
                        av[oo][0:64, :], rb[:])

        # absorb the psQK->psSC pool-transition stall (first score matmul
        # waits on the last QK-psum consumers) with V-proj work
        emit_vpair(4)
        emit_vpair(6)
        pend = None
        for qt in range(NQT):
            groups = [(kb, kb + 1) for kb in range(0, 4 * qt, 2)]
            groups.append((4 * qt, 4 * qt + 1))
            groups.append((4 * qt + 2,))
            groups.append((4 * qt + 3,))
            items = []
            for pair in range(2):
                for gi, grp in enumerate(groups):
                    for o in range(2):
                        items.append((qt, pair, grp, o, gi == 0,
                                      gi == len(groups) - 1))
            # filler units to interleave during this qt's attention:
            # V-proj for later q-tiles, out-proj of the PREVIOUS q-tile
            fillers = []
            if qt == 0:
                fillers.append(lambda: emit_vpair(8))
                fillers.append(lambda: emit_vpair(10))
            elif qt == 1:
                fillers.append(lambda: emit_vpair(12))
                fillers.append(lambda: emit_vpair(14))
            if qt >= 1:
                for tb in range(4 * (qt - 1), 4 * qt):
                    for oc in range(2):
                        fillers.append(
                            lambda tb=tb, oc=oc: emit_outproj(tb, oc))
            stride = max(1, len(items) // max(1, len(fillers)))

            for idx, (qt_, pair, grp, o, first, last) in enumerate(items):
                if first and o == 0:
                    av = []
                    for oo in range(2):
                        avt = psAV.tile([P, 512], F32, tag="av",
                                        name=f"av{pair}{oo}")
                        av.append(avt[0:65])
                    avs[(qt_, pair)] = av
                qtile = qk_tiles[("q", pair)]
                ktile = qk_tiles[("k", pair)]
                offs, ws, slots = geom(qt_, grp)
                hs = slice(64 * o, 64 * o + 64)
                sct = psSC.tile([P, 1024], F32, tag="sc", name="sc")
                for kb, off, w_, sl in zip(grp, offs, ws, slots):
                    nc.tensor.matmul(
                        sct[:, sl:sl + w_],
                        ktile[hs, kb * P:(kb + 1) * P],
                        qtile[hs, qt_ * 512 + off:(qt_ + 1) * 512],
                        start=True, stop=True)
                fd = slots[len(grp) - 1] + ws[len(grp) - 1]
                et = ex.tile([P, 1024], BF16, tag="e")
                nc.scalar.activation(
                    et[:, 0:fd], sct[:, 0:fd], EXP, scale=0.125)
                for kb, off, w_, sl in zip(grp, offs, ws, slots):
                    if kb >= 4 * qt_:
                        # causal triangle via DVE bf16 2x-mode multiply
                        nc.vector.tensor_mul(
                            et[:, sl:sl + P], et[:, sl:sl + P], tri_sb[:])
                if pend is not None:
                    emit_av(*pend)
                pend = (qt_, pair, grp, o, last, et)
                if fillers and idx % stride == stride - 1:
                    fillers.pop(0)()
            for f in fillers:
                f()
        emit_av(*pend)


def _prep_core_inputs(x, pos, Wq, Wk, Wv, Wo):
    """Per-core input dicts (host-side sharding + layout prep)."""
    bf16 = ml_dtypes.bfloat16
    inv_freq = THETA ** (-np.arange(0, HD, 2, dtype=np.float32) / HD)
    ang = pos.astype(np.float32)[:, None] * inv_freq[None, :]   # (S, 32)
    cos = np.cos(ang).astype(np.float32)                        # (S, 32)
    sin = np.sin(ang).astype(np.float32)
    p = np.arange(P)
    pairidx = (p % HD) // 2
    cosP = np.ascontiguousarray(cos[:, pairidx].T)              # (128, S)
    sgn = np.where(p % 2 == 0, -1.0, 1.0).astype(np.float32)
    sinP = np.ascontiguousarray(sin[:, pairidx].T * sgn[:, None])

    xTs = [np.ascontiguousarray(x[b].T).astype(bf16) for b in range(B)]
    maps = []
    for c in range(NCORES):
        b, g = divmod(c, NH)
        cs = slice(C * g, C * (g + 1))
        maps.append({
            "xT": xTs[b],
            "wqT": np.ascontiguousarray(Wq[cs, :].T).astype(bf16),
            "wkT": np.ascontiguousarray(Wk[cs, :].T).astype(bf16),
            "wvT": np.ascontiguousarray(Wv[cs, :].T).astype(bf16),
            "woT": np.ascontiguousarray(Wo[:, cs].T).astype(bf16),
            "cosP": cosP.astype(bf16),
            "sinP": sinP.astype(bf16),
        })
    return maps


def kernel(in_features, token_positions, Wq, Wk, Wv, Wo):
    global _NC_CACHE, LAST_RESULTS
    x = np.asarray(in_features, dtype=np.float32)
    pos = np.asarray(token_positions)
    Wq = np.asarray(Wq, dtype=np.float32)
    Wk = np.asarray(Wk, dtype=np.float32)
    Wv = np.asarray(Wv, dtype=np.float32)
    Wo = np.asarray(Wo, dtype=np.float32)

    if _NC_CACHE is None:
        _NC_CACHE = _build()
    maps = _prep_core_inputs(x, pos, Wq, Wk, Wv, Wo)
    res = run_bass_kernel_spmd(_NC_CACHE, maps, core_ids=list(range(NCORES)))
    LAST_RESULTS = res
    parts = [np.asarray(r["out"], dtype=np.float32) for r in res.results]
    outb = [parts[4 * b] + parts[4 * b + 1] + parts[4 * b + 2] + parts[4 * b + 3]
            for b in range(B)]
    return np.stack(outb).astype(np.float32)


if __name__ == "__main__":
    rng = np.random.default_rng(0)
    x = rng.standard_normal((B, S, D), dtype=np.float32)
    o = kernel(x, np.arange(S, dtype=np.int32),
               *(rng.standard_normal((D, D), dtype=np.float32) / 32
                 for _ in range(4)))
    print(o.shape, o.dtype)
